# revision 1
# baseline (speedup 1.0000x reference)
"""Trainium2 Bass kernel for GaussianKernelGCNLayer.

Reference computation (per instance b of 2048 = 8*256):
  wf[b,k,d] = sum_n w[b,n,k] * f[b,n,d]         (n=32 neighbors, k=8 kernels)
  out[b,k,o] = sum_d wf[b,k,d] * CW[k,d,o]      (d=4096, o=512)

Sharding: data-parallel over the 2048 instances -> 256 per core on 8 cores.

Per-core device algorithm (all matmul inputs bf16, fp32 PSUM accumulate):
  Phase 1: for each group g of 4 instances, stack their (32-neighbor)
    features into a [128, 4096] SBUF tile (contract dim = 4*32 = 128
    partitions) and matmul against a host-prebuilt block-diagonal
    weight tile [128, 32] whose 4 diagonal blocks are the per-instance
    [32, 8] neighbour weights.  lhsT = feature d-chunk [128, 128],
    rhs = block-diag weights -> psum [128(d), 32(bi,k)]: this yields
    wf TRANSPOSED (d on partitions) which is exactly the layout phase 2
    needs, with no on-chip transpose.
  Phase 2: for each kernel k: out[b, k*512:+512] = wf_k @ CW_k as 32
    accumulating matmuls over d-chunks; lhsT = wfT[:, c, k, mtile]
    ([128 d, 128 b] contiguous), rhs = CW[k, chunk] ([128 d, 512 o]).
"""

import os
import sys

import numpy as np

try:
    import ml_dtypes
except ImportError:  # pragma: no cover
    ml_dtypes = None

for _p in ("/opt/trn_rl_repo",):
    if _p not in sys.path:
        sys.path.insert(0, _p)

NB, NI, NN, DIN = 8, 256, 32, 4096
NK, DKO = 8, 512
NCORES = 8
BL = NB * NI // NCORES  # 256 instances per core
NGRP = BL // 4          # 64 groups of 4 instances
NCH = DIN // 128        # 32 d-chunks
BF16 = ml_dtypes.bfloat16 if ml_dtypes is not None else None

_cached_nc = None


def _build(repeat=1, phases=(1, 2)):
    from contextlib import ExitStack

    import concourse.bass as bass  # noqa: F401
    import concourse.tile as tile
    from concourse import bacc, mybir

    nc = bacc.Bacc(
        "TRN2",
        target_bir_lowering=False,
        debug=False,
        num_devices=NCORES,
    )

    f_d = nc.dram_tensor(
        "fstack", [NGRP, 128, DIN], mybir.dt.bfloat16, kind="ExternalInput"
    ).ap()
    w_d = nc.dram_tensor(
        "wblk", [NGRP, 128, 32], mybir.dt.bfloat16, kind="ExternalInput"
    ).ap()
    cw_d = nc.dram_tensor(
        "cw", [NK, DIN, DKO], mybir.dt.bfloat16, kind="ExternalInput"
    ).ap()
    out_d = nc.dram_tensor(
        "out", [BL, NK * DKO], mybir.dt.float32, kind="ExternalOutput"
    ).ap()

    with ExitStack() as ctx:
        tc = ctx.enter_context(tile.TileContext(nc))
        const_pool = ctx.enter_context(tc.tile_pool(name="const", bufs=1))
        fpool = ctx.enter_context(tc.tile_pool(name="fpool", bufs=3))
        wpool = ctx.enter_context(tc.tile_pool(name="wpool", bufs=3))
        ps1 = ctx.enter_context(tc.tile_pool(name="ps1", bufs=3, space="PSUM"))
        ps2 = ctx.enter_context(tc.tile_pool(name="ps2", bufs=4, space="PSUM"))
        wtpool = ctx.enter_context(tc.tile_pool(name="wtpool", bufs=8))
        opool = ctx.enter_context(tc.tile_pool(name="opool", bufs=4))

        # Persistent transposed wf: [128 (d%128), chunk, k, g, bi] bf16.
        # For phase 2, wfT[:, c, k, mt*32:(mt+1)*32, :] is a contiguous
        # [128, 128] block -> FWL-eligible weight loads.
        wfT = const_pool.tile(
            [128, NCH, NK, NGRP, 4], mybir.dt.bfloat16, name="wfT"
        )

        if repeat > 1:
            ctx.enter_context(tc.For_i(0, repeat, 1))

        # ---- Phase 1: wfT[d, (bi,k)] per instance-group ----
        for g in range(NGRP):
            fs = fpool.tile([128, DIN], mybir.dt.bfloat16, name="fs")
            nc.sync.dma_start(fs[:], f_d[g, :, :])
            wb = wpool.tile([128, 4, 8], mybir.dt.bfloat16, name="wb")
            nc.sync.dma_start(wb[:], w_d[g, :, :].rearrange("p (bi k) -> p bi k", k=NK))
            for h in range(2):
                pt = ps1.tile([128, 16, 4, 8], mybir.dt.float32, name="pt")
                for cc in range(16):
                    c = h * 16 + cc
                    nc.tensor.matmul(
                        pt[:, cc, :, :],
                        fs[:, c * 128 : (c + 1) * 128],
                        wb[:],
                        start=True,
                        stop=True,
                    )
                # psum [128, 16, bi, k] -> wfT[:, h*16:(h+1)*16, k, g, bi]
                nc.vector.tensor_copy(
                    wfT[:, h * 16 : (h + 1) * 16, :, g, :],
                    pt[:].rearrange("p cc bi k -> p cc k bi"),
                )

        # ---- Phase 2: out = wf @ CW, k-outer, both m-tiles per W pass ----
        for k in range(NK):
            po0 = ps2.tile([128, DKO], mybir.dt.float32, name="po0", tag="po")
            po1 = ps2.tile([128, DKO], mybir.dt.float32, name="po1", tag="po")
            pos = (po0, po1)
            for c in range(NCH):
                wt = wtpool.tile([128, DKO], mybir.dt.bfloat16, name="wt")
                nc.sync.dma_start(wt[:], cw_d[k, c * 128 : (c + 1) * 128, :])
                for mt in range(2):
                    lhs = wfT[:, c, k, mt * 32 : (mt + 1) * 32, :]
                    nc.tensor.matmul(
                        pos[mt][:],
                        lhs,
                        wt[:],
                        start=(c == 0),
                        stop=(c == NCH - 1),
                    )
            for mt in range(2):
                ot = opool.tile([128, DKO], mybir.dt.float32, name="ot")
                nc.vector.tensor_copy(ot[:], pos[mt][:])
                nc.sync.dma_start(
                    out_d[mt * 128 : (mt + 1) * 128, k * DKO : (k + 1) * DKO],
                    ot[:],
                )

    nc.compile()
    return nc


def _prep_inputs(neighbourhood_features, neighbourhood_weights, conv_weight):
    f = np.asarray(neighbourhood_features, dtype=np.float32).reshape(
        NB * NI, NN, DIN
    )
    w = np.asarray(neighbourhood_weights, dtype=np.float32).reshape(NB * NI, NN, NK)
    cw16 = np.ascontiguousarray(np.asarray(conv_weight, dtype=np.float32)).astype(
        BF16
    )
    in_maps = []
    for i in range(NCORES):
        fl = (
            f[i * BL : (i + 1) * BL]
            .reshape(NGRP, 4 * NN, DIN)
            .astype(BF16)
        )
        wl = w[i * BL : (i + 1) * BL].reshape(NGRP, 4, NN, NK)
        wblk = np.zeros((NGRP, 128, 32), dtype=np.float32)
        for bi in range(4):
            wblk[:, bi * 32 : (bi + 1) * 32, bi * 8 : (bi + 1) * 8] = wl[:, bi]
        in_maps.append(
            {
                "fstack": np.ascontiguousarray(fl),
                "wblk": wblk.astype(BF16),
                "cw": cw16,
            }
        )
    return in_maps


def _execute(neighbourhood_features, neighbourhood_weights, conv_weight, trace=False):
    global _cached_nc
    if _cached_nc is None:
        _cached_nc = _build()
    nc = _cached_nc
    from concourse import bass_utils

    in_maps = _prep_inputs(
        neighbourhood_features, neighbourhood_weights, conv_weight
    )
    res = bass_utils.run_bass_kernel_spmd(
        nc, in_maps, core_ids=list(range(NCORES)), trace=trace
    )
    outs = [np.asarray(res.results[i]["out"], dtype=np.float32) for i in range(NCORES)]
    full = np.concatenate(outs, axis=0)
    return full.reshape(NB, NI, NK * DKO), res


def kernel(neighbourhood_features, neighbourhood_weights, conv_weight):
    out, _ = _execute(
        neighbourhood_features, neighbourhood_weights, conv_weight, trace=False
    )
    return out



# revision 2
# speedup vs baseline: 1.4801x; 1.4801x over previous
"""Trainium2 Bass kernel for GaussianKernelGCNLayer.

Reference computation (per instance b of 2048 = 8*256):
  wf[b,k,d] = sum_n w[b,n,k] * f[b,n,d]         (n=32 neighbors, k=8 kernels)
  out[b,k,o] = sum_d wf[b,k,d] * CW[k,d,o]      (d=4096, o=512)

Sharding: data-parallel over the 2048 instances -> 256 per core on 8 cores.

Per-core device algorithm (all matmul inputs bf16, fp32 PSUM accumulate):
  Phase 1: for each group g of 4 instances, stack their (32-neighbor)
    features into a [128, 4096] SBUF tile (contract dim = 4*32 = 128
    partitions) and matmul against a host-prebuilt block-structured
    weight tile [128, 32] whose columns are ordered (k, bi) so the
    psum->SBUF evacuation is a monotonic strided copy (no permute).
    lhsT = feature d-chunk [128, 128] (FWL-eligible), rhs = blocked
    weights -> psum [128(d), 32(k,bi)]: wf TRANSPOSED (d on partitions),
    exactly the layout phase 2 needs, with no on-chip transpose.
  Phase 2: for each kernel k: out[b, k*512:+512] = wf_k @ CW_k as 32
    accumulating matmuls over d-chunks; lhsT = wfT[:, c, k, mtile]
    ([128 d, 128 b] contiguous), rhs = CW chunk [128 d, 512 o] sliced
    from 1 MB host-pretransposed cw DMA tiles ([128, 8, 512] each).
"""

import os
import sys

import numpy as np

try:
    import ml_dtypes
except ImportError:  # pragma: no cover
    ml_dtypes = None

for _p in ("/opt/trn_rl_repo",):
    if _p not in sys.path:
        sys.path.insert(0, _p)

NB, NI, NN, DIN = 8, 256, 32, 4096
NK, DKO = 8, 512
NCORES = 8
BL = NB * NI // NCORES  # 256 instances per core
NGRP = BL // 4          # 64 groups of 4 instances
NCH = DIN // 128        # 32 d-chunks
NQ = 4                  # cw DMA quarters per kernel (8 chunks each)
BF16 = ml_dtypes.bfloat16 if ml_dtypes is not None else None

_cached_nc = None


def _build(repeat=1, phases=(1, 2)):
    from contextlib import ExitStack

    import concourse.bass as bass  # noqa: F401
    import concourse.tile as tile
    from concourse import bacc, mybir

    nc = bacc.Bacc(
        "TRN2",
        target_bir_lowering=False,
        debug=False,
        num_devices=NCORES,
    )

    f_d = nc.dram_tensor(
        "fstack", [NGRP, 128, DIN], mybir.dt.bfloat16, kind="ExternalInput"
    ).ap()
    w_d = nc.dram_tensor(
        "wblk", [128, NGRP, 32], mybir.dt.bfloat16, kind="ExternalInput"
    ).ap()
    cw_d = nc.dram_tensor(
        "cwt", [128, NK, NCH, DKO], mybir.dt.bfloat16, kind="ExternalInput"
    ).ap()
    out_d = nc.dram_tensor(
        "out", [BL, NK * DKO], mybir.dt.float32, kind="ExternalOutput"
    ).ap()

    with ExitStack() as ctx:
        tc = ctx.enter_context(tile.TileContext(nc))
        const_pool = ctx.enter_context(tc.tile_pool(name="const", bufs=1))
        fpool = ctx.enter_context(tc.tile_pool(name="fpool", bufs=3))
        wpool = ctx.enter_context(tc.tile_pool(name="wpool", bufs=1))
        ps1 = ctx.enter_context(tc.tile_pool(name="ps1", bufs=4, space="PSUM"))
        ps2 = ctx.enter_context(tc.tile_pool(name="ps2", bufs=4, space="PSUM"))
        cwpool = ctx.enter_context(tc.tile_pool(name="cwpool", bufs=2))
        opool = ctx.enter_context(tc.tile_pool(name="opool", bufs=2))

        # Persistent transposed wf: [128 (d%128), chunk, k, g, bi] bf16.
        # For phase 2, wfT[:, c, k, mt*32:(mt+1)*32, :] is a contiguous
        # [128, 128] block -> FWL-eligible weight loads.
        wfT = const_pool.tile(
            [128, NCH, NK, NGRP, 4], mybir.dt.bfloat16, name="wfT"
        )

        if repeat > 1:
            ctx.enter_context(tc.For_i(0, repeat, 1))

        # Blocked neighbour weights for ALL groups: one 512 KB DMA.
        wb = wpool.tile([128, NGRP, 32], mybir.dt.bfloat16, name="wb")
        nc.sync.dma_start(wb[:], w_d[:, :, :])

        # ---- Phase 1: wfT[d, (k,bi)] per instance-group ----
        if 1 in phases:
            for g in range(NGRP):
                fs = fpool.tile([128, DIN], mybir.dt.bfloat16, name="fs")
                nc.sync.dma_start(fs[:], f_d[g, :, :])
                for h in range(2):
                    pt = ps1.tile([128, 16, 8, 4], mybir.dt.float32, name="pt")
                    for cc in range(16):
                        c = h * 16 + cc
                        nc.tensor.matmul(
                            pt[:, cc, :, :],
                            fs[:, c * 128 : (c + 1) * 128],
                            wb[:, g, :],
                            start=True,
                            stop=True,
                        )
                    # psum [128, 16, k, bi] -> wfT[:, h*16:(h+1)*16, :, g, :]
                    # (same index order on both sides: plain strided copy)
                    nc.vector.tensor_copy(
                        wfT[:, h * 16 : (h + 1) * 16, :, g, :],
                        pt[:],
                    )

        # ---- Phase 2: out = wf @ CW, k-outer, both m-tiles per W pass ----
        if 2 in phases:
            for k in range(NK):
                po0 = ps2.tile([128, DKO], mybir.dt.float32, name="po0", tag="po")
                po1 = ps2.tile([128, DKO], mybir.dt.float32, name="po1", tag="po")
                pos = (po0, po1)
                for q in range(NQ):
                    wt = cwpool.tile([128, NCH // NQ, DKO], mybir.dt.bfloat16, name="wt")
                    nc.sync.dma_start(
                        wt[:], cw_d[:, k, q * (NCH // NQ) : (q + 1) * (NCH // NQ), :]
                    )
                    for cc in range(NCH // NQ):
                        c = q * (NCH // NQ) + cc
                        for mt in range(2):
                            lhs = wfT[:, c, k, mt * 32 : (mt + 1) * 32, :]
                            nc.tensor.matmul(
                                pos[mt][:],
                                lhs,
                                wt[:, cc, :],
                                start=(c == 0),
                                stop=(c == NCH - 1),
                            )
                for mt in range(2):
                    ot = opool.tile([128, DKO], mybir.dt.float32, name="ot")
                    nc.vector.tensor_copy(ot[:], pos[mt][:])
                    nc.sync.dma_start(
                        out_d[mt * 128 : (mt + 1) * 128, k * DKO : (k + 1) * DKO],
                        ot[:],
                    )

    nc.compile()
    return nc


def _prep_inputs(neighbourhood_features, neighbourhood_weights, conv_weight):
    f = np.asarray(neighbourhood_features, dtype=np.float32).reshape(
        NB * NI, NN, DIN
    )
    w = np.asarray(neighbourhood_weights, dtype=np.float32).reshape(NB * NI, NN, NK)
    # cwt[p, k, c, o] = cw[k, c*128+p, o]  (shared across cores)
    cw = np.asarray(conv_weight, dtype=np.float32).reshape(NK, NCH, 128, DKO)
    cwt = np.ascontiguousarray(cw.transpose(2, 0, 1, 3)).astype(BF16)
    in_maps = []
    for i in range(NCORES):
        fl = (
            f[i * BL : (i + 1) * BL]
            .reshape(NGRP, 4 * NN, DIN)
            .astype(BF16)
        )
        wl = w[i * BL : (i + 1) * BL].reshape(NGRP, 4, NN, NK)
        # wblk[bi*32+n, g, k, bi] = wl[g, bi, n, k]; rhs column = k*4+bi
        wblk = np.zeros((128, NGRP, NK, 4), dtype=np.float32)
        for bi in range(4):
            wblk[bi * 32 : (bi + 1) * 32, :, :, bi] = wl[:, bi].transpose(1, 0, 2)
        in_maps.append(
            {
                "fstack": np.ascontiguousarray(fl),
                "wblk": wblk.reshape(128, NGRP, 32).astype(BF16),
                "cwt": cwt,
            }
        )
    return in_maps


def _execute(neighbourhood_features, neighbourhood_weights, conv_weight, trace=False):
    global _cached_nc
    if _cached_nc is None:
        _cached_nc = _build()
    nc = _cached_nc
    from concourse import bass_utils

    in_maps = _prep_inputs(
        neighbourhood_features, neighbourhood_weights, conv_weight
    )
    res = bass_utils.run_bass_kernel_spmd(
        nc, in_maps, core_ids=list(range(NCORES)), trace=trace
    )
    outs = [np.asarray(res.results[i]["out"], dtype=np.float32) for i in range(NCORES)]
    full = np.concatenate(outs, axis=0)
    return full.reshape(NB, NI, NK * DKO), res


def kernel(neighbourhood_features, neighbourhood_weights, conv_weight):
    out, _ = _execute(
        neighbourhood_features, neighbourhood_weights, conv_weight, trace=False
    )
    return out


# revision 8
# speedup vs baseline: 1.8406x; 1.2435x over previous
"""Trainium2 Bass kernel for GaussianKernelGCNLayer.

Reference computation (per instance b of 2048 = 8*256):
  wf[b,k,d] = sum_n w[b,n,k] * f[b,n,d]         (n=32 neighbors, k=8 kernels)
  out[b,k,o] = sum_d wf[b,k,d] * CW[k,d,o]      (d=4096, o=512)

Sharding: data-parallel over the 2048 instances -> 256 per core on 8 cores.

Per-core device algorithm (all matmul inputs bf16, fp32 PSUM accumulate):
  Phase 1: for each group g of 4 instances, stack their (32-neighbor)
    features into a [128, 4096] SBUF tile (contract dim = 4*32 = 128
    partitions) and matmul against a host-prebuilt block-structured
    weight tile [128, 32] whose columns are ordered (k, bi) so the
    psum->SBUF evacuation is a monotonic strided copy (no permute).
    lhsT = feature d-chunk [128, 128] (FWL-eligible), rhs = blocked
    weights -> psum [128(d), 32(k,bi)]: wf TRANSPOSED (d on partitions),
    exactly the layout phase 2 needs, with no on-chip transpose.
  Phase 2: for each kernel k: out[b, k*512:+512] = wf_k @ CW_k as 32
    accumulating matmuls over d-chunks; lhsT = wfT[:, c, k, mtile]
    ([128 d, 128 b] contiguous), rhs = CW chunk [128 d, 512 o] sliced
    from 1 MB host-pretransposed cw DMA tiles ([128, 8, 512] each).
"""

import os
import sys

import numpy as np

try:
    import ml_dtypes
except ImportError:  # pragma: no cover
    ml_dtypes = None

for _p in ("/opt/trn_rl_repo",):
    if _p not in sys.path:
        sys.path.insert(0, _p)

NB, NI, NN, DIN = 8, 256, 32, 4096
NK, DKO = 8, 512
NCORES = 8
BL = NB * NI // NCORES  # 256 instances per core
NGRP = BL // 4          # 64 groups of 4 instances
NCH = DIN // 128        # 32 d-chunks
NQ = 4                  # cw DMA quarters per kernel (8 chunks each)
BF16 = ml_dtypes.bfloat16 if ml_dtypes is not None else None

_cached_nc = None


def _build(repeat=1, phases=(1, 2), p1_mms=16):
    from contextlib import ExitStack

    import concourse.bass as bass  # noqa: F401
    import concourse.tile as tile
    from concourse import bacc, mybir

    nc = bacc.Bacc(
        "TRN2",
        target_bir_lowering=False,
        debug=False,
        num_devices=NCORES,
    )

    f_d = nc.dram_tensor(
        "fstack", [NGRP, 128, DIN], mybir.dt.bfloat16, kind="ExternalInput"
    ).ap()
    w_d = nc.dram_tensor(
        "wblk", [128, NGRP, 32], mybir.dt.bfloat16, kind="ExternalInput"
    ).ap()
    cw_d = nc.dram_tensor(
        "cwt", [128, NK, NCH, DKO], mybir.dt.bfloat16, kind="ExternalInput"
    ).ap()
    out_d = nc.dram_tensor(
        "out", [BL, NK * DKO], mybir.dt.float32, kind="ExternalOutput"
    ).ap()

    with ExitStack() as ctx:
        tc = ctx.enter_context(tile.TileContext(nc))
        const_pool = ctx.enter_context(tc.tile_pool(name="const", bufs=1))
        fpool = ctx.enter_context(tc.tile_pool(name="fpool", bufs=3))
        wpool = ctx.enter_context(tc.tile_pool(name="wpool", bufs=1))
        ps1 = ctx.enter_context(tc.tile_pool(name="ps1", bufs=4, space="PSUM"))
        ps2 = ctx.enter_context(tc.tile_pool(name="ps2", bufs=4, space="PSUM"))
        cwpool = ctx.enter_context(tc.tile_pool(name="cwpool", bufs=3))
        opool = ctx.enter_context(tc.tile_pool(name="opool", bufs=2))

        # Persistent transposed wf: [128 (d%128), chunk, k, g, bi] bf16.
        # For phase 2, wfT[:, c, k, mt*32:(mt+1)*32, :] is a contiguous
        # [128, 128] block -> FWL-eligible weight loads.
        wfT = const_pool.tile(
            [128, NCH, NK, NGRP, 4], mybir.dt.bfloat16, name="wfT"
        )

        if 1 not in phases:
            # timing-ablation only: give wfT a writer so Tile allocates it
            nc.vector.memset(wfT[:, :, :, 0, :], 0.0)

        if repeat > 1:
            ctx.enter_context(tc.For_i(0, repeat, 1))

        # Blocked neighbour weights for ALL groups: one 512 KB DMA.
        wb = wpool.tile([128, NGRP, 32], mybir.dt.bfloat16, name="wb")
        nc.sync.dma_start(wb[:], w_d[:, :, :])

        # ---- Phase 1: wfT[d, (k,bi)] per instance-group ----
        if 1 in phases:
            for g in range(NGRP):
                fs = fpool.tile([128, DIN], mybir.dt.bfloat16, name="fs")
                nc.sync.dma_start(fs[:], f_d[g, :, :])
                for h in range(2):
                    pt = ps1.tile([128, 16, 8, 4], mybir.dt.float32, name="pt")
                    for cc in range(p1_mms):
                        c = h * 16 + cc
                        nc.tensor.matmul(
                            pt[:, cc, :, :],
                            fs[:, c * 128 : (c + 1) * 128],
                            wb[:, g, :],
                            start=True,
                            stop=True,
                        )
                    # psum [128, 16, k, bi] -> wfT[:, h*16:(h+1)*16, :, g, :]
                    # (same index order on both sides: plain strided copy)
                    nc.vector.tensor_copy(
                        wfT[:, h * 16 : h * 16 + p1_mms, :, g, :],
                        pt[:, :p1_mms, :, :],
                    )

        # ---- Phase 2: out = wf @ CW, k-outer, both m-tiles per W pass ----
        if 2 in phases:
            for k in range(NK):
                po0 = ps2.tile([128, DKO], mybir.dt.float32, name="po0", tag="po")
                po1 = ps2.tile([128, DKO], mybir.dt.float32, name="po1", tag="po")
                pos = (po0, po1)
                for q in range(NQ):
                    wt = cwpool.tile([128, NCH // NQ, DKO], mybir.dt.bfloat16, name="wt")
                    nc.sync.dma_start(
                        wt[:], cw_d[:, k, q * (NCH // NQ) : (q + 1) * (NCH // NQ), :]
                    )
                    for cc in range(NCH // NQ):
                        c = q * (NCH // NQ) + cc
                        for mt in range(2):
                            lhs = wfT[:, c, k, mt * 32 : (mt + 1) * 32, :]
                            nc.tensor.matmul(
                                pos[mt][:],
                                lhs,
                                wt[:, cc, :],
                                start=(c == 0),
                                stop=(c == NCH - 1),
                            )
                for mt in range(2):
                    ot = opool.tile([128, DKO], mybir.dt.float32, name="ot")
                    nc.vector.tensor_copy(ot[:], pos[mt][:])
                    # ACT HWDGE queue: keeps the SP queue free for input DMAs
                    nc.scalar.dma_start(
                        out_d[mt * 128 : (mt + 1) * 128, k * DKO : (k + 1) * DKO],
                        ot[:],
                    )

    nc.compile()
    return nc


def _prep_inputs(neighbourhood_features, neighbourhood_weights, conv_weight):
    f = np.asarray(neighbourhood_features, dtype=np.float32).reshape(
        NB * NI, NN, DIN
    )
    w = np.asarray(neighbourhood_weights, dtype=np.float32).reshape(NB * NI, NN, NK)
    # cwt[p, k, c, o] = cw[k, c*128+p, o]  (shared across cores)
    cw = np.asarray(conv_weight, dtype=np.float32).reshape(NK, NCH, 128, DKO)
    cwt = np.ascontiguousarray(cw.transpose(2, 0, 1, 3)).astype(BF16)
    in_maps = []
    for i in range(NCORES):
        fl = (
            f[i * BL : (i + 1) * BL]
            .reshape(NGRP, 4 * NN, DIN)
            .astype(BF16)
        )
        wl = w[i * BL : (i + 1) * BL].reshape(NGRP, 4, NN, NK)
        # wblk[bi*32+n, g, k, bi] = wl[g, bi, n, k]; rhs column = k*4+bi
        wblk = np.zeros((128, NGRP, NK, 4), dtype=np.float32)
        for bi in range(4):
            wblk[bi * 32 : (bi + 1) * 32, :, :, bi] = wl[:, bi].transpose(1, 0, 2)
        in_maps.append(
            {
                "fstack": np.ascontiguousarray(fl),
                "wblk": wblk.reshape(128, NGRP, 32).astype(BF16),
                "cwt": cwt,
            }
        )
    return in_maps


def _execute(neighbourhood_features, neighbourhood_weights, conv_weight, trace=False):
    global _cached_nc
    if _cached_nc is None:
        _cached_nc = _build()
    nc = _cached_nc
    from concourse import bass_utils

    in_maps = _prep_inputs(
        neighbourhood_features, neighbourhood_weights, conv_weight
    )
    res = bass_utils.run_bass_kernel_spmd(
        nc, in_maps, core_ids=list(range(NCORES)), trace=trace
    )
    outs = [np.asarray(res.results[i]["out"], dtype=np.float32) for i in range(NCORES)]
    full = np.concatenate(outs, axis=0)
    return full.reshape(NB, NI, NK * DKO), res


def kernel(neighbourhood_features, neighbourhood_weights, conv_weight):
    out, _ = _execute(
        neighbourhood_features, neighbourhood_weights, conv_weight, trace=False
    )
    return out
